# revision 3
# baseline (speedup 1.0000x reference)
"""Trainium2 Bass kernel for nn_CombinedLoss (robot trajectory + phase loss).

v2: bf16 plane-major (SoA) layout. Host re-packs each robot component and
phase logit as a flat [N]-plane; the device computes all loss partial sums
in bf16 with f32 accumulators, balanced across Act/DVE/Pool; host reduces
in f64 and applies exact boundary corrections (same flat-sequence scheme
as v1 - the per-plane flat stream equals the per-component flat stream).
"""
import sys, os

for _p in (os.path.expanduser("~/.axon_site/_ro/trn_rl_repo"), "/opt/trn_rl_repo"):
    if os.path.isdir(_p) and _p not in sys.path:
        sys.path.insert(0, _p)

import numpy as np
import ml_dtypes
import concourse.bass as bass
import concourse.tile as tile
from concourse import bacc, mybir, bass_utils
from concourse.alu_op_type import AluOpType as OP

F32 = mybir.dt.float32
BF16 = mybir.dt.bfloat16
AF = mybir.ActivationFunctionType
BF = ml_dtypes.bfloat16

# ---- problem constants (hardcoded) ----
B, T, D = 256, 8192, 12
NCORES = 8
BC = B // NCORES              # 32 batches per core
N = BC * T                    # 262144 frames per core
MAX_SPEED = 10.0

# robot plane layout
FP_R = N // 128               # 2048 frames per partition
NP_R = N + 256                # padded plane stride (zeros at tail)
NSLAB = 4                     # 3 planes per slab

# phase chunking
FP_P = FP_R // 2              # 1024 frames per partition per chunk
NP_P = N + 256

# strip columns
SMSE = 0                      # 4
SVV = 4                       # 4
SCROSS = 8                    # 4
SPEN = 12                     # 2
SLSE = 14
SX0 = 15                      # 2
SXJ1 = 17
SXJ2 = 18
SCA = 19
SCB = 20
SCO = 21
NCOLS = 22

# engine flags for tunable stages: 'act' | 'dve' | 'pool'
CFG = {
    "d2": ["act"] * NSLAB,        # act: Square+accum ; dve: TT+ts
    "s2": ["pool"] * NSLAB,       # pool/dve TT adds
    "se": ["dve", "dve"],         # TT adds per phase chunk
    "cross": ["dve"] * NSLAB,
    "msq": "act",
}


def build():
    nc = bacc.Bacc("TRN2", target_bir_lowering=False, debug=False)

    xr = nc.dram_tensor("xr", [12 * NP_R], BF16, kind="ExternalInput")
    gn = nc.dram_tensor("gn", [12 * N], BF16, kind="ExternalInput")
    ph = nc.dram_tensor("ph", [3 * NP_P], BF16, kind="ExternalInput")
    gtf = nc.dram_tensor("gtf", [N], BF16, kind="ExternalInput")
    out = nc.dram_tensor("partials", [128, NCOLS], F32, kind="ExternalOutput").ap()

    W = FP_R                      # 2048
    WE = W + 2                    # x window
    WV = W + 1                    # vel window
    WP = FP_P                     # 1024
    WPE = WP + 1

    with tile.TileContext(nc) as tc:
        with tc.tile_pool(name="hold", bufs=1) as hold:
            strip = hold.tile([128, NCOLS], F32)
            s2hold = hold.tile([128, NSLAB, W], BF16)
            seh = hold.tile([128, 2, WP], BF16)
            mh = hold.tile([128, 2, WPE], BF16)
            idxh = hold.tile([128, 2, WPE], BF16)
            d10h = hold.tile([128, 2, WP], BF16)
            d21h = hold.tile([128, 2, WP], BF16)
            g1h = hold.tile([128, 2, WP], BF16)
            g2h = hold.tile([128, 2, WP], BF16)

            with tc.tile_pool(name="rp", bufs=2) as rp, \
                 tc.tile_pool(name="jp", bufs=2) as jp, \
                 tc.tile_pool(name="pp", bufs=2) as pp:

                def robot_slab(s):
                    xt = rp.tile([128, 3, WE], BF16)
                    nc.sync.dma_start(
                        xt[:], bass.AP(xr, 3 * s * NP_R, [[W, 128], [NP_R, 3], [1, WE]]))
                    v = rp.tile([128, 3, WV], BF16)
                    nc.vector.tensor_tensor(v[:], xt[:, :, 1:WE], xt[:, :, 0:WV],
                                            OP.subtract)
                    # MSE: accumulate -g into x in place, then square+accum
                    nc.gpsimd.dma_start(
                        xt[:, :, 0:W], bass.AP(gn, 3 * s * N, [[W, 128], [N, 3], [1, W]]),
                        accum_op=OP.add)
                    if CFG["d2"][s] == "act":
                        d2j = jp.tile([128, 3, W], BF16, tag="jbig")
                        nc.scalar.activation(d2j[:], xt[:, :, 0:W], AF.Square,
                                             accum_out=strip[:, SMSE + s:SMSE + s + 1])
                    else:
                        d2j = jp.tile([128, 3, W], BF16, tag="jbig")
                        nc.vector.tensor_tensor(d2j[:], xt[:, :, 0:W], xt[:, :, 0:W],
                                                OP.mult)
                        dsj = jp.tile([128, 3, W], BF16, tag="jbig")
                        nc.vector.tensor_scalar(
                            out=dsj[:], in0=d2j[:], scalar1=1.0, scalar2=0.0,
                            op0=OP.mult, op1=OP.add,
                            accum_out=strip[:, SMSE + s:SMSE + s + 1])
                    # cross: v_n * v_{n+1}
                    crj = jp.tile([128, 3, W], BF16, tag="jbig")
                    nc.vector.tensor_tensor(crj[:], v[:, :, 0:W], v[:, :, 1:WV],
                                            OP.mult)
                    csj = jp.tile([128, 3, W], BF16, tag="jbig")
                    nc.vector.tensor_scalar(
                        out=csj[:], in0=crj[:], scalar1=1.0, scalar2=0.0,
                        op0=OP.mult, op1=OP.add,
                        accum_out=strip[:, SCROSS + s:SCROSS + s + 1])
                    # V2 in place over v[:, :, 0:W]
                    nc.vector.tensor_tensor(v[:, :, 0:W], v[:, :, 0:W], v[:, :, 0:W],
                                            OP.mult)
                    # s2 = plane0 + plane1 + plane2
                    s2a = jp.tile([128, W], BF16, tag="jmed")
                    eng = nc.gpsimd if CFG["s2"][s] == "pool" else nc.vector
                    eng.tensor_tensor(s2a[:], v[:, 0, 0:W], v[:, 1, 0:W], OP.add)
                    eng.tensor_tensor(s2hold[:, s, :], s2a[:], v[:, 2, 0:W], OP.add)
                    # svv = sum(s2)
                    svj = jp.tile([128, W], BF16, tag="jmed")
                    nc.vector.tensor_scalar(
                        out=svj[:], in0=s2hold[:, s, :], scalar1=1.0, scalar2=0.0,
                        op0=OP.mult, op1=OP.add,
                        accum_out=strip[:, SVV + s:SVV + s + 1])

                def phase_chunk(c):
                    pt = pp.tile([128, 3, WPE], BF16)
                    nc.sync.dma_start(
                        pt[:], bass.AP(ph, c * 128 * WP, [[WP, 128], [NP_P, 3], [1, WPE]]))
                    gtt = pp.tile([128, WP], BF16)
                    nc.sync.dma_start(
                        gtt[:], bass.AP(gtf, c * 128 * WP, [[WP, 128], [1, WP]]))
                    # sum of x0 over valid frames
                    x0j = jp.tile([128, WP], BF16, tag="jmed")
                    nc.vector.tensor_scalar(
                        out=x0j[:], in0=pt[:, 0, 0:WP], scalar1=1.0, scalar2=0.0,
                        op0=OP.mult, op1=OP.add,
                        accum_out=strip[:, SX0 + c:SX0 + c + 1])
                    nc.vector.tensor_tensor(d10h[:, c, :], pt[:, 1, 0:WP],
                                            pt[:, 0, 0:WP], OP.subtract)
                    nc.vector.tensor_tensor(d21h[:, c, :], pt[:, 2, 0:WP],
                                            pt[:, 1, 0:WP], OP.subtract)
                    nc.vector.tensor_scalar(out=g1h[:, c, :], in0=gtt[:], scalar1=1.0,
                                            scalar2=0.0, op0=OP.min, op1=OP.add)
                    nc.vector.tensor_scalar(out=g2h[:, c, :], in0=gtt[:], scalar1=-1.0,
                                            scalar2=0.0, op0=OP.add, op1=OP.max)
                    # argmax (first-match) of the 3 logits, incl overlap col
                    b1 = pp.tile([128, WPE], BF16)
                    t1 = pp.tile([128, WPE], BF16)
                    t2 = pp.tile([128, WPE], BF16)
                    nc.vector.tensor_tensor(b1[:], pt[:, 1, :], pt[:, 0, :], OP.is_gt)
                    nc.vector.tensor_tensor(t1[:], pt[:, 0, :], pt[:, 1, :], OP.max)
                    nc.vector.tensor_tensor(t2[:], pt[:, 2, :], t1[:], OP.is_gt)
                    nc.vector.tensor_tensor(mh[:, c, :], t1[:], pt[:, 2, :], OP.max)
                    nc.vector.tensor_scalar(out=t1[:], in0=b1[:], scalar1=-1.0,
                                            scalar2=2.0, op0=OP.mult, op1=OP.add)
                    nc.vector.tensor_tensor(t2[:], t2[:], t1[:], OP.mult)
                    nc.vector.tensor_tensor(idxh[:, c, :], t2[:], b1[:], OP.add)
                    # exp in place over valid cols, then se = e0+e1+e2
                    nc.scalar.activation(pt[:, :, 0:WP], pt[:, :, 0:WP], AF.Exp)
                    sea = pp.tile([128, WP], BF16)
                    eng = nc.gpsimd if CFG["se"][c] == "pool" else nc.vector
                    eng.tensor_tensor(sea[:], pt[:, 0, 0:WP], pt[:, 1, 0:WP], OP.add)
                    eng.tensor_tensor(seh[:, c, :], sea[:], pt[:, 2, 0:WP], OP.add)

                # interleave robot slabs and phase chunks for DMA overlap
                robot_slab(0)
                phase_chunk(0)
                robot_slab(1)
                robot_slab(2)
                phase_chunk(1)
                robot_slab(3)

                # ---- speed tail: sqrt, relu shift, square, accum (2 halves) ----
                for h in range(2):
                    sl = s2hold[:, 2 * h:2 * h + 2, :]
                    st = jp.tile([128, 2, W], BF16, tag="jmed2")
                    nc.scalar.activation(st[:], sl, AF.Sqrt)
                    pt_ = jp.tile([128, 2, W], BF16, tag="jmed2")
                    nc.vector.tensor_scalar(out=pt_[:], in0=st[:], scalar1=MAX_SPEED,
                                            scalar2=-MAX_SPEED, op0=OP.max, op1=OP.add)
                    pj = jp.tile([128, 2, W], BF16, tag="jmed2")
                    nc.vector.tensor_tensor(pj[:], pt_[:], pt_[:], OP.mult)
                    psj = jp.tile([128, 2, W], BF16, tag="jmed2")
                    nc.vector.tensor_scalar(
                        out=psj[:], in0=pj[:], scalar1=1.0, scalar2=0.0,
                        op0=OP.mult, op1=OP.add,
                        accum_out=strip[:, SPEN + h:SPEN + h + 1])

                # ---- phase tail ----
                lnj = jp.tile([128, 2, WP], BF16, tag="jmed")
                nc.scalar.activation(lnj[:], seh[:], AF.Ln,
                                     accum_out=strip[:, SLSE:SLSE + 1])
                jj = jp.tile([128, 2, WP], BF16, tag="jmed")
                nc.vector.tensor_tensor(jj[:], d10h[:], g1h[:], OP.mult)
                jsj = jp.tile([128, 2, WP], BF16, tag="jmed")
                nc.vector.tensor_scalar(
                    out=jsj[:], in0=jj[:], scalar1=1.0, scalar2=0.0,
                    op0=OP.mult, op1=OP.add, accum_out=strip[:, SXJ1:SXJ1 + 1])
                jj2 = jp.tile([128, 2, WP], BF16, tag="jmed")
                nc.vector.tensor_tensor(jj2[:], d21h[:], g2h[:], OP.mult)
                jsj2 = jp.tile([128, 2, WP], BF16, tag="jmed")
                nc.vector.tensor_scalar(
                    out=jsj2[:], in0=jj2[:], scalar1=1.0, scalar2=0.0,
                    op0=OP.mult, op1=OP.add, accum_out=strip[:, SXJ2:SXJ2 + 1])
                # coherence tail (reuse dead hold tiles: dd->d10h, A->g2h,
                # B->seh, mask->d21h, msq->g1h)
                nc.vector.tensor_tensor(d10h[:], idxh[:, :, 1:WPE], idxh[:, :, 0:WP],
                                        OP.subtract)
                nc.vector.tensor_scalar(out=g2h[:], in0=d10h[:], scalar1=-0.5,
                                        scalar2=0.0, op0=OP.is_le, op1=OP.add,
                                        accum_out=strip[:, SCA:SCA + 1])
                nc.vector.tensor_scalar(out=seh[:], in0=d10h[:], scalar1=1.5,
                                        scalar2=0.0, op0=OP.is_ge, op1=OP.add,
                                        accum_out=strip[:, SCB:SCB + 1])
                nc.vector.tensor_tensor(d21h[:], g2h[:], seh[:], OP.add)
                if CFG["msq"] == "act":
                    nc.scalar.activation(g1h[:], mh[:, :, 1:WPE], AF.Square)
                else:
                    nc.vector.tensor_tensor(g1h[:], mh[:, :, 1:WPE],
                                            mh[:, :, 1:WPE], OP.mult)
                cjj = jp.tile([128, 2, WP], BF16, tag="jmed")
                nc.vector.tensor_tensor(cjj[:], d21h[:], g1h[:], OP.mult)
                csj2 = jp.tile([128, 2, WP], BF16, tag="jmed")
                nc.vector.tensor_scalar(
                    out=csj2[:], in0=cjj[:], scalar1=1.0, scalar2=0.0,
                    op0=OP.mult, op1=OP.add, accum_out=strip[:, SCO:SCO + 1])

            nc.sync.dma_start(out, strip[:])
    nc.compile()
    return nc


_NC_CACHE = None


def _get_nc():
    global _NC_CACHE
    if _NC_CACHE is None:
        _NC_CACHE = build()
    return _NC_CACHE


def _prep_core(xs, ps, gs, ts):
    """Per-core input map. xs,gs: [BC,T,D] f32; ps: [BC,T,3] f32; ts: [BC,T] i32."""
    xr = np.zeros((12, NP_R), BF)
    xr[:, :N] = xs.reshape(N, D).T.astype(BF)
    gn = np.ascontiguousarray((-gs).reshape(N, D).T.astype(BF))
    ph = np.zeros((3, NP_P), BF)
    ph[:, :N] = ps.reshape(N, 3).T.astype(BF)
    return {
        "xr": xr.reshape(-1),
        "gn": gn.reshape(-1),
        "ph": ph.reshape(-1),
        "gtf": ts.astype(BF).reshape(-1),
    }


def _argmax3(a):
    """first-match argmax over last axis of [..., 3], matching the device."""
    b1 = a[..., 1] > a[..., 0]
    c2 = a[..., 2] > np.maximum(a[..., 0], a[..., 1])
    return b1 + c2 * (2.0 - b1)


def _host_finish(strips, pred_robot, pred_phase):
    """strips: list of [128, NCOLS] per core. Returns f32 scalar total loss."""
    S = np.stack([s.astype(np.float64).sum(axis=0) for s in strips])  # [8, NCOLS]
    tot = S.sum(axis=0)
    mse_sum = tot[SMSE:SMSE + NSLAB].sum()
    svv = tot[SVV:SVV + NSLAB].sum()
    scross = tot[SCROSS:SCROSS + NSLAB].sum()
    sspeed = tot[SPEN:SPEN + 2].sum()
    slse = tot[SLSE]
    sx0 = tot[SX0:SX0 + 2].sum()
    sxg1 = tot[SXJ1]
    sxg2 = tot[SXJ2]
    scnt = tot[SCA] + tot[SCB]
    sco = tot[SCO]

    # ---- boundary corrections (f64, tiny) ----
    svv_c = 0.0; sspeed_c = 0.0; cross_c = 0.0; edge_sum = 0.0
    cnt_c = 0.0; co_c = 0.0
    for ci in range(NCORES):
        Xb = pred_robot[ci * BC:(ci + 1) * BC].astype(np.float64)  # [BC,T,D]
        # invalid flat vels at n = k*T-1, k=1..BC
        vbad = np.empty((BC, D))
        vbad[:BC - 1] = Xb[1:, 0] - Xb[:-1, T - 1]
        vbad[BC - 1] = -Xb[BC - 1, T - 1]           # pad-zero edge
        svv_c += (vbad ** 2).sum()
        s2b = (vbad.reshape(BC, 4, 3) ** 2).sum(-1)
        pen = np.maximum(np.sqrt(s2b) - MAX_SPEED, 0.0)
        sspeed_c += (pen ** 2).sum()
        # invalid cross products: v_{nk-1}*vbad + vbad*v_{nk+1}
        vprev = Xb[:, T - 1] - Xb[:, T - 2]          # [BC,D] last valid vel
        vnext = Xb[:, 1] - Xb[:, 0]                  # first valid vel
        cross_c += (vprev * vbad).sum()
        cross_c += (vbad[:BC - 1] * vnext[1:]).sum()
        # per-batch edge vels for the acc identity
        edge_sum += (vnext ** 2).sum() + (vprev ** 2).sum()
        # phase coherence corrections at pair t = k*T-1
        Pb = pred_phase[ci * BC:(ci + 1) * BC].astype(np.float64)  # [BC,T,3]
        a = Pb[:, T - 1]                              # logits at t
        b = np.zeros_like(a)
        b[:BC - 1] = Pb[1:, 0]                        # logits at t+1 (pad zero last)
        ua = _argmax3(a)
        ub = _argmax3(b)
        dd = ub - ua
        mask = (dd <= -0.5) + (dd >= 1.5)
        cnt_c += mask.sum()
        co_c += (mask * b.max(-1) ** 2).sum()

    svv_t = svv - svv_c
    cross_t = scross - cross_c
    sspeed_t = sspeed - sspeed_c
    acc_sum = 2.0 * svv_t - edge_sum - 2.0 * cross_t
    cnt_t = scnt - cnt_c
    co_t = sco - co_c

    robot_loss = mse_sum / (B * T * D)
    xgt = sx0 + sxg1 + sxg2
    phase_loss = (slse - xgt) / (B * T)
    coherence = (co_t / max(cnt_t, 1.0)) if cnt_t > 0 else 0.0
    speed_loss = 5.0 * sspeed_t / (B * (T - 1) * 4)
    vel_loss = svv_t / (B * (T - 1) * D)
    acc_loss = acc_sum / (B * (T - 2) * D)
    total = (robot_loss + phase_loss + 10.0 * coherence + speed_loss
             + 0.05 * vel_loss + 0.01 * acc_loss)
    return np.asarray(total, dtype=np.float32)


def kernel(pred_robot, pred_phase, gt_robot, gt_phase):
    nc = _get_nc()
    in_maps = []
    for c in range(NCORES):
        sl = slice(c * BC, (c + 1) * BC)
        in_maps.append(_prep_core(pred_robot[sl], pred_phase[sl],
                                  gt_robot[sl], gt_phase[sl]))
    res = bass_utils.run_bass_kernel_spmd(nc, in_maps, core_ids=list(range(NCORES)))
    strips = [res.results[c]["partials"] for c in range(NCORES)]
    return _host_finish(strips, pred_robot, pred_phase)
